# revision 1
# baseline (speedup 1.0000x reference)
"""Trainium2 Bass kernel for nn_AttentionBlock (sparse_attention).

Full-input contract: kernel(**inputs) takes the complete tensors and returns
the complete [4, 512, 512] output. Internally shards over 8 NeuronCores as
(batch, i-half): core c handles batch c//2, query rows (c%2)*256 ..+256.

Self-contained: hardcodes all shapes; no sibling imports.
"""

import sys

if "/opt/trn_rl_repo" not in sys.path:
    sys.path.insert(0, "/opt/trn_rl_repo")

from contextlib import ExitStack

import numpy as np

import concourse.bass as bass
import concourse.tile as tile
from concourse import bacc, mybir
from concourse.bass_utils import run_bass_kernel_spmd
from concourse.masks import make_identity

F32 = mybir.dt.float32
F32R = mybir.dt.float32r
U8 = mybir.dt.uint8
U32 = mybir.dt.uint32
AF = mybir.ActivationFunctionType
ALU = mybir.AluOpType
AX = mybir.AxisListType

NEG = -1.0e30

# Full-problem constants
B, L_FULL, H, NH = 4, 512, 512, 8
DK = H // NH  # 64
CIN, CHID = 53, 32  # repr MLP dims
N_CORES = 8


def build_program(L, LI, has_bq, has_bk, has_bv, has_bo, has_r2b, r2b_vals,
                  trace_sim=False):
    """One-core program: attention block over LI query rows, L context.

    L multiple of 128; LI multiple of 16 (and of 128 when >= 128).
    r2b_vals: python floats baked as immediates when has_r2b.
    """
    assert L % 128 == 0 and LI % 16 == 0
    NJB = L // 128            # j blocks (also k-chunks of v / wT)
    NPAIR = LI // 2           # i-row pairs (layer-1 packing)
    IBS = min(128, LI)        # i-block size for attention tiles
    NIB = LI // IBS           # i-blocks
    NHC = H // 128            # h chunks (4)
    scale = float(1.0 / np.sqrt(DK))

    nc = bacc.Bacc()

    xfull = nc.dram_tensor("xfull", [L, H], F32, kind="ExternalInput")
    xq_d = nc.dram_tensor("xq", [LI, H], F32, kind="ExternalInput")
    refc = nc.dram_tensor("refc", [LI, L, CIN], F32, kind="ExternalInput")
    masku8 = nc.dram_tensor("masku8", [L, 1], U8, kind="ExternalInput")
    maskpi = nc.dram_tensor("maskpi", [L, 1], U8, kind="ExternalInput")
    ioff_d = nc.dram_tensor("ioff", [LI, 1], U8, kind="ExternalInput")
    wq_d = nc.dram_tensor("wq", [H, H], F32, kind="ExternalInput")
    wk_d = nc.dram_tensor("wk", [H, H], F32, kind="ExternalInput")
    wv_d = nc.dram_tensor("wv", [H, H], F32, kind="ExternalInput")
    wo_d = nc.dram_tensor("wo", [H, H], F32, kind="ExternalInput")
    bq_d = nc.dram_tensor("bq", [H, 1], F32, kind="ExternalInput")
    bk_d = nc.dram_tensor("bk", [H, 1], F32, kind="ExternalInput")
    bv_d = nc.dram_tensor("bv", [H, 1], F32, kind="ExternalInput")
    bo_d = nc.dram_tensor("bo", [H, 1], F32, kind="ExternalInput")
    r1w_d = nc.dram_tensor("r1w", [CIN, CHID], F32, kind="ExternalInput")
    bd1_d = nc.dram_tensor("bd1h", [2 * CIN, 2 * CHID], F32, kind="ExternalInput")
    bd2_d = nc.dram_tensor("bd2h", [128, 4 * NH], F32, kind="ExternalInput")
    r1b_d = nc.dram_tensor("r1b", [CHID, 1], F32, kind="ExternalInput")
    r2w_d = nc.dram_tensor("r2w", [CHID, NH], F32, kind="ExternalInput")
    lng_d = nc.dram_tensor("lng", [H, 1], F32, kind="ExternalInput")
    lnb_d = nc.dram_tensor("lnb", [H, 1], F32, kind="ExternalInput")
    y_d = nc.dram_tensor("y", [LI, H], F32, kind="ExternalOutput")

    with tile.TileContext(nc, trace_sim=trace_sim) as tc, ExitStack() as ctx:
        P = ctx.enter_context(tc.tile_pool(name="persist", bufs=1))
        pc2 = ctx.enter_context(tc.tile_pool(name="c2", bufs=3))
        ptrs = ctx.enter_context(tc.tile_pool(name="trs", bufs=3))
        phid = ctx.enter_context(tc.tile_pool(name="hid", bufs=2))
        pea = ctx.enter_context(tc.tile_pool(name="ea", bufs=2))
        per = ctx.enter_context(tc.tile_pool(name="er", bufs=2))
        pwts = ctx.enter_context(tc.tile_pool(name="wts", bufs=2))
        pry = ctx.enter_context(tc.tile_pool(name="ry", bufs=2))
        psc = ctx.enter_context(tc.tile_pool(name="sc", bufs=4))
        # PSUM pools — 2+1+2+2+1 = 8 banks
        pp_main = ctx.enter_context(tc.tile_pool(name="ppmain", bufs=2, space="PSUM"))
        pp_tr = ctx.enter_context(tc.tile_pool(name="pptr", bufs=2, space="PSUM"))
        pp_l1 = ctx.enter_context(tc.tile_pool(name="ppl1", bufs=2, space="PSUM"))
        pp_l2 = ctx.enter_context(tc.tile_pool(name="ppl2", bufs=1, space="PSUM"))
        pp_wav = ctx.enter_context(tc.tile_pool(name="ppwav", bufs=1, space="PSUM"))

        # ---------- constants / weights ----------
        ident = P.tile([128, 128], F32, tag="ident")
        make_identity(nc, ident)

        bd1 = P.tile([2 * CIN, 2 * CHID], F32R, tag="bd1")  # [106, 64]
        nc.sync.dma_start(out=bd1, in_=bd1_d[:, :].bitcast(F32R))
        bd2 = P.tile([128, 4 * NH], F32R, tag="bd2")  # [128, 32]
        nc.sync.dma_start(out=bd2, in_=bd2_d[:, :].bitcast(F32R))

        r1b4 = P.tile([128, 1], F32, tag="r1b4")
        for g in range(4):
            nc.sync.dma_start(out=r1b4[32 * g : 32 * g + CHID, :], in_=r1b_d[:, :])

        w_sb = {}
        for nm, d in (("wq", wq_d), ("wk", wk_d), ("wv", wv_d), ("wo", wo_d)):
            for kk in range(NHC):
                t = P.tile([128, H], F32R, tag=f"{nm}{kk}")
                nc.sync.dma_start(
                    out=t, in_=d[128 * kk : 128 * (kk + 1), :].bitcast(F32R)
                )
                w_sb[(nm, kk)] = t

        bias_sb = {}
        for nm, d, has in (("bq", bq_d, has_bq), ("bk", bk_d, has_bk), ("bv", bv_d, has_bv)):
            if has:
                for kk in range(NHC):
                    t = P.tile([128, 1], F32, tag=f"{nm}{kk}")
                    nc.sync.dma_start(out=t, in_=d[128 * kk : 128 * (kk + 1), :])
                    bias_sb[(nm, kk)] = t

        # ---------- mask tiles ----------
        # j axis is globally permuted: column C <-> j = 4*(C%128) + C//128
        # (from the contiguous refCov load layout). Consistent across
        # kT/v/scores/mask, so softmax/AV are unaffected.
        mjb_u8 = P.tile([128, L], U8, tag="mjbu8")
        nc.sync.dma_start(
            out=mjb_u8, in_=bass.AP(tensor=maskpi, offset=0, ap=[[0, 128], [1, L]])
        )
        mjb = P.tile([128, L], F32, tag="mjb")
        nc.vector.tensor_copy(out=mjb, in_=mjb_u8)
        negt = P.tile([128, L], F32, tag="negt")
        nc.gpsimd.memset(negt, NEG)
        eps_t = P.tile([128, 1], F32, tag="eps_t")
        nc.gpsimd.memset(eps_t, 1e-5)

        inval = []
        for ib in range(NIB):
            miu = P.tile([IBS, 1], U8, tag=f"miu{ib}")
            nc.sync.dma_start(out=miu, in_=ioff_d[IBS * ib : IBS * (ib + 1), :])
            mif = P.tile([IBS, 1], F32, tag=f"mif{ib}")
            nc.vector.tensor_copy(out=mif, in_=miu)
            iv = P.tile([IBS, L], U32, tag=f"inval{ib}")
            # (mjb * mi) == 0 -> 1 where invalid, 0 where valid
            nc.vector.tensor_scalar(
                out=iv, in0=mjb[0:IBS, :], scalar1=mif, scalar2=0.0,
                op0=ALU.mult, op1=ALU.is_equal,
            )
            inval.append(iv)

        g_bc = P.tile([128, H], F32, tag="g_bc")
        nc.sync.dma_start(
            out=g_bc, in_=bass.AP(tensor=lng_d, offset=0, ap=[[0, 128], [1, H]])
        )
        b_bc = P.tile([128, H], F32, tag="b_bc")
        nc.sync.dma_start(
            out=b_bc, in_=bass.AP(tensor=lnb_d, offset=0, ap=[[0, 128], [1, H]])
        )
        bo_bc = None
        if has_bo:
            bo_bc = P.tile([128, H], F32, tag="bo_bc")
            nc.sync.dma_start(
                out=bo_bc, in_=bass.AP(tensor=bo_d, offset=0, ap=[[0, 128], [1, H]])
            )

        # ---------- x loads, transposes ----------
        xf = []
        for t in range(NJB):
            # row p of tile t = x row j = NJB*p + t (the pi-permuted j order)
            xt = P.tile([128, H], F32, tag=f"xf{t}")
            nc.sync.dma_start(out=xt, in_=xfull[t : L : NJB, :])
            xf.append(xt)
        xq_sb = []
        for ib in range(NIB):
            xt = P.tile([IBS, H], F32, tag=f"xq{ib}")
            nc.sync.dma_start(out=xt, in_=xq_d[IBS * ib : IBS * (ib + 1), :])
            xq_sb.append(xt)

        xT = []  # [h-chunk][128, L] — transposed full x (for k, v)
        for hc in range(NHC):
            ps = pp_main.tile([128, L], F32, tag="big")
            for jt in range(NJB):
                nc.tensor.transpose(
                    out=ps[:, 128 * jt : 128 * (jt + 1)],
                    in_=xf[jt][:, 128 * hc : 128 * (hc + 1)],
                    identity=ident,
                )
            xs = P.tile([128, L], F32, tag=f"xT{hc}")
            nc.scalar.copy(out=xs.bitcast(F32R), in_=ps)
            xT.append(xs)
        xqT = []  # [h-chunk][128, LI] — transposed xq (for q)
        for hc in range(NHC):
            ps = pp_main.tile([128, LI], F32, tag="big")
            for ib in range(NIB):
                nc.tensor.transpose(
                    out=ps[:, IBS * ib : IBS * (ib + 1)],
                    in_=xq_sb[ib][:, 128 * hc : 128 * (hc + 1)],
                    identity=ident[0:IBS, 0:IBS],
                )
            xs = P.tile([128, LI], F32, tag=f"xqT{hc}")
            nc.scalar.copy(out=xs.bitcast(F32R), in_=ps)
            xqT.append(xs)

        # ---------- q/k/v projections ----------
        qT = []
        for t in range(NHC):
            ps = pp_main.tile([128, LI], F32, tag="big")
            for kk in range(NHC):
                nc.tensor.matmul(
                    out=ps,
                    lhsT=w_sb[("wq", kk)][:, 128 * t : 128 * (t + 1)],
                    rhs=xqT[kk].bitcast(F32R), start=(kk == 0), stop=(kk == NHC - 1),
                )
            s = P.tile([128, LI], F32, tag=f"qT{t}")
            if has_bq:
                nc.scalar.activation(out=s.bitcast(F32R), in_=ps, func=AF.Identity, bias=bias_sb[("bq", t)])
            else:
                nc.scalar.copy(out=s.bitcast(F32R), in_=ps)
            qT.append(s)
        kT = []
        for t in range(NHC):
            ps = pp_main.tile([128, L], F32, tag="big")
            for kk in range(NHC):
                nc.tensor.matmul(
                    out=ps,
                    lhsT=w_sb[("wk", kk)][:, 128 * t : 128 * (t + 1)],
                    rhs=xT[kk].bitcast(F32R), start=(kk == 0), stop=(kk == NHC - 1),
                )
            s = P.tile([128, L], F32, tag=f"kT{t}")
            if has_bk:
                nc.scalar.activation(out=s.bitcast(F32R), in_=ps, func=AF.Identity, bias=bias_sb[("bk", t)])
            else:
                nc.scalar.copy(out=s.bitcast(F32R), in_=ps)
            kT.append(s)
        v_sb = []
        for t in range(NJB):
            ps = pp_main.tile([128, H], F32, tag="big")
            for kk in range(NHC):
                nc.tensor.matmul(
                    out=ps, lhsT=xT[kk][:, 128 * t : 128 * (t + 1)].bitcast(F32R),
                    rhs=w_sb[("wv", kk)],
                    start=(kk == 0), stop=(kk == NHC - 1),
                )
            s = P.tile([128, H], F32, tag=f"v{t}")
            nc.scalar.copy(out=s.bitcast(F32R), in_=ps)  # bv folded into avT evac
            v_sb.append(s)

        # persistent per-head numerator holders for the ref path
        refS = [
            P.tile([IBS, NH, L], F32, tag=f"refS{ib}", name=f"refS{ib}")
            for ib in range(NIB)
        ]
        aoT = [P.tile([128, LI], F32, tag=f"aoT{t}", name=f"aoT{t}") for t in range(NHC)]

        # ---------- main per-i-block phases ----------
        for ib in range(NIB):
            # ---- repr-MLP over this i-block's rows ----
            p0 = (IBS * ib) // 2
            for rr in range(IBS // 2):
                r = p0 + rr
                i0r, i1r = 2 * r, 2 * r + 1
                c2t = pc2.tile([128, 2, NJB, CIN], F32, tag="c2t")
                # one fully-contiguous DMA per pair: partition p holds
                # j-rows {NJB*p + k} — the pi-permuted j order.
                nc.sync.dma_start(
                    out=c2t,
                    in_=refc[i0r : i0r + 2].rearrange(
                        "i (p k) c -> p i k c", k=NJB
                    ),
                )
                # repack on idle GPSIMD into transpose-friendly layout
                # (matmul data operand needs a single free dim)
                c2p = pc2.tile([128, NJB, 2, CIN], F32, tag="c2p")
                nc.gpsimd.tensor_copy(
                    out=c2p, in_=c2t.rearrange("p i k c -> p k i c")
                )
                trp = pp_tr.tile([2 * CIN, L], F32, tag="tr")
                for jb in range(NJB):
                    # contiguous [128, 106]: rows 0:53 = i even, 53:106 = i odd
                    nc.tensor.transpose(
                        out=trp[:, 128 * jb : 128 * (jb + 1)],
                        in_=c2p[:, jb, :, :].rearrange("p i c -> p (i c)"),
                        identity=ident,
                    )
                trs = ptrs.tile([2 * CIN, L], F32, tag="trs")
                if rr % 2 == 0:
                    nc.scalar.copy(out=trs.bitcast(F32R), in_=trp)
                else:
                    nc.vector.tensor_copy(out=trs.bitcast(F32R), in_=trp)
                # f32r matmul psum outputs must start at partition 0:
                # one [64, L] psum tile per pair; the relu evacuation packs
                # pairs into hid halves instead.
                l1p = pp_l1.tile([64, L], F32, tag="l1")
                nc.tensor.matmul(
                    out=l1p, lhsT=bd1, rhs=trs.bitcast(F32R), start=True, stop=True,
                )
                if rr % 2 == 0:
                    hid = phid.tile([128, L], F32, tag="hid")
                    nc.scalar.activation(
                        out=hid[0:64, :].bitcast(F32R), in_=l1p,
                        func=AF.Relu, bias=r1b4[0:64, :],
                    )
                else:
                    # relu on DVE: (x + b) max 0
                    nc.vector.tensor_scalar(
                        out=hid[64:128, :].bitcast(F32R), in0=l1p,
                        scalar1=r1b4[64:128, :], scalar2=0.0,
                        op0=ALU.add, op1=ALU.max,
                    )
                if rr % 2 == 1:
                    qq = rr // 2  # quad (4 i-rows) index within i-block
                    l2p = pp_l2.tile([32, L], F32, tag="l2")
                    nc.tensor.matmul(
                        out=l2p, lhsT=bd2, rhs=hid.bitcast(F32R),
                        start=True, stop=True,
                    )
                    if qq % 4 == 0:
                        l2s = ptrs.tile([128, L], F32, tag="l2s")
                    q4 = qq % 4
                    if q4 % 2 == 0:
                        nc.scalar.copy(out=l2s[32 * q4 : 32 * q4 + 32, :], in_=l2p)
                    else:
                        nc.vector.tensor_copy(out=l2s[32 * q4 : 32 * q4 + 32, :], in_=l2p)
                    if q4 == 3 or rr == IBS // 2 - 1:
                        mrow0 = 16 * (qq // 4)  # row offset within i-block
                        nrows = 4 * (q4 + 1)
                        # de-interleave: partition-strided reads, one per head
                        for nh in range(NH):
                            nc.sync.dma_start(
                                out=refS[ib][mrow0 : mrow0 + nrows, nh, :],
                                in_=l2s[nh : 8 * nrows : 8, :],
                            )

            # ---- attention + ref softmax + combine + AV per head ----
            for nh in range(NH):
                t, s = nh // 2, nh % 2
                sp = pp_main.tile([IBS, L], F32, tag="big")
                nc.tensor.matmul(
                    out=sp,
                    lhsT=qT[t][64 * s : 64 * s + 64, IBS * ib : IBS * (ib + 1)].bitcast(F32R),
                    rhs=kT[t][64 * s : 64 * s + 64, :].bitcast(F32R),
                    start=True, stop=True,
                )
                nc.vector.copy_predicated(out=sp, mask=inval[ib], data=negt[0:IBS, :])
                rmax = psc.tile([IBS, 1], F32, tag="rmax")
                nc.vector.tensor_reduce(out=rmax, in_=sp, axis=AX.X, op=ALU.max)
                nm8 = psc.tile([IBS, 1], F32, tag="nm8")
                nc.vector.tensor_scalar_mul(out=nm8, in0=rmax, scalar1=-scale)
                ea_t = pea.tile([IBS, L], F32, tag="ea")
                sa = psc.tile([IBS, 1], F32, tag="sa")
                nc.scalar.activation(
                    out=ea_t, in_=sp, func=AF.Exp, bias=nm8, scale=scale, accum_out=sa
                )

                rt = refS[ib][:, nh, :]
                if has_r2b:
                    nc.vector.tensor_scalar_add(out=rt, in0=rt, scalar1=float(r2b_vals[nh]))
                nc.vector.copy_predicated(out=rt, mask=inval[ib], data=negt[0:IBS, :])
                rmax2 = psc.tile([IBS, 1], F32, tag="rmax2")
                nc.vector.tensor_reduce(out=rmax2, in_=rt, axis=AX.X, op=ALU.max)
                nm2 = psc.tile([IBS, 1], F32, tag="nm2")
                nc.vector.tensor_scalar_mul(out=nm2, in0=rmax2, scalar1=-1.0)
                er_t = per.tile([IBS, L], F32, tag="er")
                sr = psc.tile([IBS, 1], F32, tag="sr")
                nc.scalar.activation(
                    out=er_t, in_=rt, func=AF.Exp, bias=nm2, scale=1.0, accum_out=sr
                )

                isa = psc.tile([IBS, 1], F32, tag="isa")
                nc.vector.reciprocal(out=isa, in_=sa)
                isr = psc.tile([IBS, 1], F32, tag="isr")
                nc.vector.reciprocal(out=isr, in_=sr)
                # w = ea/sa + er/sr (0.5 factor folded into avT evac scale)
                nc.vector.tensor_scalar_mul(out=ea_t, in0=ea_t, scalar1=isa)
                nc.vector.scalar_tensor_tensor(
                    out=ea_t, in0=er_t, scalar=isr, in1=ea_t, op0=ALU.mult, op1=ALU.add
                )

                wtp = pp_wav.tile([128, NJB * IBS], F32, tag="wav")
                for k in range(NJB):
                    nc.tensor.transpose(
                        out=wtp[:, IBS * k : IBS * (k + 1)],
                        in_=ea_t[:, 128 * k : 128 * (k + 1)],
                        identity=ident[0:IBS, 0:IBS],
                    )
                wts = pwts.tile([128, NJB * IBS], F32, tag="wts")
                nc.scalar.copy(out=wts.bitcast(F32R), in_=wtp)

                avp = pp_wav.tile([64, IBS], F32, tag="wav")
                for k in range(NJB):
                    nc.tensor.matmul(
                        out=avp,
                        lhsT=v_sb[k][:, 64 * nh : 64 * nh + 64].bitcast(F32R),
                        rhs=wts[:, IBS * k : IBS * (k + 1)].bitcast(F32R),
                        start=(k == 0), stop=(k == NJB - 1),
                    )
                if has_bv:
                    nc.scalar.activation(
                        out=aoT[t][64 * s : 64 * s + 64, IBS * ib : IBS * (ib + 1)].bitcast(F32R),
                        in_=avp, func=AF.Identity, scale=0.5,
                        bias=bias_sb[("bv", t)][64 * s : 64 * s + 64, :],
                    )
                else:
                    nc.scalar.activation(
                        out=aoT[t][64 * s : 64 * s + 64, IBS * ib : IBS * (ib + 1)].bitcast(F32R),
                        in_=avp, func=AF.Copy, bias=0.0, scale=0.5,
                    )

            # ---- output projection + residual + layernorm ----
            pp = pp_main.tile([IBS, H], F32, tag="big")
            for kk in range(NHC):
                nc.tensor.matmul(
                    out=pp, lhsT=aoT[kk][:, IBS * ib : IBS * (ib + 1)].bitcast(F32R),
                    rhs=w_sb[("wo", kk)],
                    start=(kk == 0), stop=(kk == NHC - 1),
                )
            r_t = pry.tile([IBS, H], F32, tag="rt")
            nc.vector.scalar_tensor_tensor(
                out=r_t, in0=pp, scalar=1.0, in1=xq_sb[ib], op0=ALU.mult, op1=ALU.add
            )
            if has_bo:
                nc.vector.tensor_add(out=r_t, in0=r_t, in1=bo_bc[0:IBS, :])
            stats = psc.tile([IBS, 6], F32, tag="stats")
            nc.vector.bn_stats(out=stats, in_=r_t)
            mv = psc.tile([IBS, 2], F32, tag="mv")
            nc.vector.bn_aggr(out=mv, in_=stats)
            stdv = psc.tile([IBS, 1], F32, tag="stdv")
            nc.scalar.activation(out=stdv, in_=mv[:, 1:2], func=AF.Sqrt, bias=eps_t[0:IBS, :])
            rstd = psc.tile([IBS, 1], F32, tag="rstd")
            nc.vector.reciprocal(out=rstd, in_=stdv)
            nc.vector.tensor_scalar(
                out=r_t, in0=r_t, scalar1=mv[:, 0:1], scalar2=rstd,
                op0=ALU.subtract, op1=ALU.mult,
            )
            y_t = pry.tile([IBS, H], F32, tag="yt")
            nc.vector.tensor_mul(out=y_t, in0=r_t, in1=g_bc[0:IBS, :])
            nc.vector.tensor_add(out=y_t, in0=y_t, in1=b_bc[0:IBS, :])
            nc.scalar.dma_start(out=y_d[IBS * ib : IBS * (ib + 1), :], in_=y_t)

    nc.compile()  # Bacc legalization: ≤1 sync wait per instruction, etc.
    return nc


def _make_bd1(r1w):
    bd1 = np.zeros((2 * CIN, 2 * CHID), np.float32)
    bd1[0:CIN, 0:CHID] = r1w
    bd1[CIN : 2 * CIN, CHID : 2 * CHID] = r1w
    return bd1


def _make_bd2(r2w):
    bd2 = np.zeros((128, 4 * NH), np.float32)
    for g in range(4):
        bd2[32 * g : 32 * g + CHID, NH * g : NH * (g + 1)] = r2w
    return bd2


_PROG_CACHE = {}


def _get_program(L, LI, flags, r2b_vals):
    key = (L, LI, flags)
    if key not in _PROG_CACHE:
        _PROG_CACHE[key] = build_program(L, LI, *flags, r2b_vals)
    return _PROG_CACHE[key]


def make_in_maps(x, mask, refCov, wq, bq, wk, bk, wv, bv, wo, bo,
                 r1w, r1b, r2w, r2b, ln_g, ln_b, n_cores=N_CORES, LI=None):
    Bc, L, Hc = x.shape
    if LI is None:
        LI = (Bc * L) // n_cores
    f = np.float32
    shared = {
        "wq": np.ascontiguousarray(wq, f), "wk": np.ascontiguousarray(wk, f),
        "wv": np.ascontiguousarray(wv, f), "wo": np.ascontiguousarray(wo, f),
        "bq": np.ascontiguousarray(bq, f).reshape(Hc, 1),
        "bk": np.ascontiguousarray(bk, f).reshape(Hc, 1),
        "bv": np.ascontiguousarray(bv, f).reshape(Hc, 1),
        "bo": np.ascontiguousarray(bo, f).reshape(Hc, 1),
        "r1w": np.ascontiguousarray(r1w, f),
        "bd1h": _make_bd1(np.asarray(r1w, f)),
        "bd2h": _make_bd2(np.asarray(r2w, f)),
        "r1b": np.ascontiguousarray(r1b, f).reshape(CHID, 1),
        "r2w": np.ascontiguousarray(r2w, f),
        "lng": np.ascontiguousarray(ln_g, f).reshape(Hc, 1),
        "lnb": np.ascontiguousarray(ln_b, f).reshape(Hc, 1),
    }
    per_batch = L // LI  # cores per batch
    in_maps = []
    for c in range(n_cores):
        b, half = c // per_batch, c % per_batch
        i0 = half * LI
        m = dict(shared)
        m["xfull"] = np.ascontiguousarray(x[b], f)
        m["xq"] = np.ascontiguousarray(x[b, i0 : i0 + LI], f)
        m["refc"] = np.ascontiguousarray(refCov[b, i0 : i0 + LI], f)
        m["masku8"] = np.ascontiguousarray(mask[b].astype(np.uint8).reshape(L, 1))
        njb = L // 128
        pidx = (np.arange(L) % 128) * njb + np.arange(L) // 128
        m["maskpi"] = np.ascontiguousarray(mask[b][pidx].astype(np.uint8).reshape(L, 1))
        m["ioff"] = np.ascontiguousarray(
            mask[b, i0 : i0 + LI].astype(np.uint8).reshape(LI, 1)
        )
        in_maps.append(m)
    return in_maps, per_batch, LI


def kernel(x, mask, refCov, wq, bq, wk, bk, wv, bv, wo, bo,
           r1w, r1b, r2w, r2b, ln_g, ln_b, trace=False):
    x = np.asarray(x)
    Bc, L, Hc = x.shape
    flags = (
        bool(np.any(bq)), bool(np.any(bk)), bool(np.any(bv)), bool(np.any(bo)),
        bool(np.any(r2b)),
    )
    in_maps, per_batch, LI = make_in_maps(
        x, mask, refCov, wq, bq, wk, bk, wv, bv, wo, bo,
        r1w, r1b, r2w, r2b, ln_g, ln_b,
    )
    nc = _get_program(L, LI, flags, [float(v) for v in np.asarray(r2b).ravel()])
    res = run_bass_kernel_spmd(nc, in_maps, core_ids=list(range(N_CORES)), trace=trace)
    out = np.empty((Bc, L, Hc), np.float32)
    for c in range(N_CORES):
        b, half = c // per_batch, c % per_batch
        out[b, half * LI : (half + 1) * LI] = res.results[c]["y"]
    if trace:
        return out, res
    return out



# revision 15
# speedup vs baseline: 2.1271x; 2.1271x over previous
"""Trainium2 Bass kernel for nn_AttentionBlock (sparse_attention).

Full-input contract: kernel(**inputs) takes the complete tensors and returns
the complete [4, 512, 512] output. Internally shards over 8 NeuronCores as
(batch, i-half): core c handles batch c//2, query rows (c%2)*256 ..+256.

bf16 matmul pipeline. Host pre-processing: refCov is pre-permuted into the
on-chip tile layout and cast to bf16 (halves HBM traffic and removes the
on-chip repack entirely); weights / x / small constants are host-cast to
bf16 where used as matmul operands. Softmax runs without max-subtraction
(logits are O(1)); j-masking is folded into the score matmul (K=1 bias-row
accumulate) and the repr-MLP L2 evacuation; invalid-i rows are fixed with
one per-partition tensor_scalar; layernorm rsqrt runs on DVE (magic +
Newton) so ACT keeps a single function table. refc streams in 16-row
chunks with 2-chunk DMA prefetch; deinterleave DMAs issue from the (idle)
GPSIMD engine so they never block the SP DMA queue.

Self-contained: hardcodes all shapes; no sibling imports.
"""

import sys

if "/opt/trn_rl_repo" not in sys.path:
    sys.path.insert(0, "/opt/trn_rl_repo")

from collections import deque
from contextlib import ExitStack

import ml_dtypes
import numpy as np

import concourse.bass as bass
import concourse.tile as tile
from concourse import bacc, mybir
from concourse.bass_utils import run_bass_kernel_spmd
from concourse.masks import make_identity

F32 = mybir.dt.float32
BF16 = mybir.dt.bfloat16
I32 = mybir.dt.int32
AF = mybir.ActivationFunctionType
ALU = mybir.AluOpType
AX = mybir.AxisListType

BF = ml_dtypes.bfloat16
NEG = -1.0e30

B, L_FULL, H, NH = 4, 512, 512, 8
DK = H // NH  # 64
CIN, CHID = 53, 32
N_CORES = 8
PRE = 2  # refc chunk DMA prefetch depth


def build_program(L, LI, has_bq, has_bk, has_bv, has_bo, has_r2b, r2b_vals,
                  trace_sim=False):
    """One-core program: attention block over LI query rows, L context."""
    assert L % 128 == 0 and LI % 16 == 0
    NJB = L // 128            # j blocks
    IBS = min(128, LI)        # i-block size for attention tiles
    NIB = LI // IBS           # i-blocks
    NHC = H // 128            # h chunks (4)
    NCH = LI // 16            # refc chunks (16 i-rows each)
    CPI = IBS // 16           # chunks per i-block
    CB_F = 8 * NJB * 2 * CIN  # free elems per c2b partition row
    BIGF = max(L, H)          # "big" psum tile free size
    scale = float(1.0 / np.sqrt(DK))

    nc = bacc.Bacc()

    xb_d = nc.dram_tensor("xb", [L, H], BF16, kind="ExternalInput")
    xqb_d = nc.dram_tensor("xqb", [LI, H], BF16, kind="ExternalInput")
    xq_d = nc.dram_tensor("xq", [LI, H], F32, kind="ExternalInput")
    refcb_d = nc.dram_tensor("refcb", [NCH * 128, CB_F], BF16, kind="ExternalInput")
    wqkvo_d = nc.dram_tensor("wqkvo", [H, 4 * H], BF16, kind="ExternalInput")
    bqkvo_d = nc.dram_tensor("bqkvo", [H, 4], F32, kind="ExternalInput")
    bd1_d = nc.dram_tensor("bd1h", [2 * CIN, 2 * CHID], BF16, kind="ExternalInput")
    bd2_d = nc.dram_tensor("bd2h", [128, 4 * NH], BF16, kind="ExternalInput")
    r1b4_d = nc.dram_tensor("r1b4h", [128, 1], F32, kind="ExternalInput")
    jbias_d = nc.dram_tensor("jbias", [1, L], BF16, kind="ExternalInput")
    mbias_d = nc.dram_tensor("mbias", [L, 1], F32, kind="ExternalInput")
    miof_d = nc.dram_tensor("miof", [LI, 2], F32, kind="ExternalInput")
    lng_d = nc.dram_tensor("lng", [H, 1], F32, kind="ExternalInput")
    lnb_d = nc.dram_tensor("lnb", [H, 1], F32, kind="ExternalInput")
    y_d = nc.dram_tensor("y", [LI, H], F32, kind="ExternalOutput")

    with tile.TileContext(nc, trace_sim=trace_sim) as tc, ExitStack() as ctx:
        P = ctx.enter_context(tc.tile_pool(name="persist", bufs=1))
        pc2b = ctx.enter_context(tc.tile_pool(name="c2b", bufs=PRE + 1))
        ptrs = ctx.enter_context(tc.tile_pool(name="trs", bufs=3))
        phid = ctx.enter_context(tc.tile_pool(name="hid", bufs=2))
        pl2s = ctx.enter_context(tc.tile_pool(name="l2s", bufs=3))
        pea = ctx.enter_context(tc.tile_pool(name="ea", bufs=3))
        per = ctx.enter_context(tc.tile_pool(name="er", bufs=3))
        pwts = ctx.enter_context(tc.tile_pool(name="wts", bufs=3))
        pry = ctx.enter_context(tc.tile_pool(name="ry", bufs=2))
        psc = ctx.enter_context(tc.tile_pool(name="sc", bufs=6))
        # PSUM pools (2KB banks): big 2 + tr 2 + l1 2 + l2 1 + wav 1 = 8
        pp_big = ctx.enter_context(tc.tile_pool(name="ppbig", bufs=2, space="PSUM"))
        pp_tr = ctx.enter_context(tc.tile_pool(name="pptr", bufs=2, space="PSUM"))
        pp_l1 = ctx.enter_context(tc.tile_pool(name="ppl1", bufs=2, space="PSUM"))
        pp_l2 = ctx.enter_context(tc.tile_pool(name="ppl2", bufs=1, space="PSUM"))
        pp_wav = ctx.enter_context(tc.tile_pool(name="ppwav", bufs=1, space="PSUM"))

        # ---------- phase 0: identity + x loads first on the DMA queue ------
        ident = P.tile([128, 128], F32, tag="ident")
        make_identity(nc, ident)
        identb = P.tile([128, 128], BF16, tag="identb")
        nc.vector.tensor_copy(out=identb, in_=ident)

        xf = []
        for t in range(NJB):
            # row p of tile t = x row j = NJB*p + t (pi-permuted j order)
            xt = P.tile([128, H], BF16, tag=f"xf{t}")
            nc.sync.dma_start(out=xt, in_=xb_d[t : L : NJB, :])
            xf.append(xt)
        xq_sb = []
        xqb_sb = []
        for ib in range(NIB):
            xt = P.tile([IBS, H], BF16, tag=f"xqb{ib}")
            nc.sync.dma_start(out=xt, in_=xqb_d[IBS * ib : IBS * (ib + 1), :])
            xqb_sb.append(xt)
            xt = P.tile([IBS, H], F32, tag=f"xq{ib}")
            nc.sync.dma_start(out=xt, in_=xq_d[IBS * ib : IBS * (ib + 1), :])
            xq_sb.append(xt)

        # ---------- weights + small constants (all pre-cast on host) --------
        wbig = []
        for kk in range(NHC):
            t = P.tile([128, 4 * H], BF16, tag=f"wbig{kk}")
            nc.sync.dma_start(out=t, in_=wqkvo_d[128 * kk : 128 * (kk + 1), :])
            wbig.append(t)

        def w_sb(nm, kk):
            wi = ("wq", "wk", "wv", "wo").index(nm)
            return wbig[kk][:, H * wi : H * (wi + 1)]

        bd1b = P.tile([2 * CIN, 2 * CHID], BF16, tag="bd1b")
        nc.sync.dma_start(out=bd1b, in_=bd1_d[:, :])
        bd2b = P.tile([128, 4 * NH], BF16, tag="bd2b")
        nc.sync.dma_start(out=bd2b, in_=bd2_d[:, :])
        r1b4 = P.tile([128, 1], F32, tag="r1b4")
        nc.sync.dma_start(out=r1b4, in_=r1b4_d[:, :])

        jbias_b = P.tile([1, L], BF16, tag="jbias_b")
        nc.sync.dma_start(out=jbias_b, in_=jbias_d[:, :])
        ones1b = P.tile([1, 128], BF16, tag="ones1b")
        nc.gpsimd.memset(ones1b, 1.0)

        mbias128 = P.tile([128, L], F32, tag="mbias128")
        nc.sync.dma_start(
            out=mbias128, in_=bass.AP(tensor=mbias_d, offset=0, ap=[[0, 128], [1, L]])
        )
        miof = []
        for ib in range(NIB):
            t = P.tile([IBS, 2], F32, tag=f"miof{ib}")
            nc.sync.dma_start(out=t, in_=miof_d[IBS * ib : IBS * (ib + 1), :])
            miof.append(t)

        g_bc = P.tile([128, H], F32, tag="g_bc")
        nc.sync.dma_start(
            out=g_bc, in_=bass.AP(tensor=lng_d, offset=0, ap=[[0, 128], [1, H]])
        )
        b_bc = P.tile([128, H], F32, tag="b_bc")
        nc.sync.dma_start(
            out=b_bc, in_=bass.AP(tensor=lnb_d, offset=0, ap=[[0, 128], [1, H]])
        )
        bo_bc = None
        if has_bo:
            bo_d2 = nc.dram_tensor("bo2", [H, 1], F32, kind="ExternalInput")
            bo_bc = P.tile([128, H], F32, tag="bo_bc")
            nc.sync.dma_start(
                out=bo_bc, in_=bass.AP(tensor=bo_d2, offset=0, ap=[[0, 128], [1, H]])
            )
        r2b128 = None
        if has_r2b:
            r2b128 = P.tile([128, 1], F32, tag="r2b128")
            r2bh_d = nc.dram_tensor("r2b128h", [128, 1], F32, kind="ExternalInput")
            nc.sync.dma_start(out=r2b128, in_=r2bh_d[:, :])

        bias_sb = {}
        if has_bq or has_bk or has_bv:
            bqkvo = P.tile([128, NHC, 4], F32, tag="bqkvo")
            nc.sync.dma_start(
                out=bqkvo, in_=bqkvo_d[:, :].rearrange("(c p) b -> p c b", p=128)
            )
            for wi, nm in enumerate(("bq", "bk", "bv")):
                for kk in range(NHC):
                    bias_sb[(nm, kk)] = bqkvo[:, kk, wi : wi + 1]

        # ---------- refc chunk prefetch machinery ----------
        c2b_q = deque()

        def issue_c2b(c):
            t = pc2b.tile([128, 8, NJB, 2, CIN], BF16, tag="c2b", name="c2b")
            nc.sync.dma_start(
                out=t.rearrange("p a k i c -> p (a k i c)"),
                in_=refcb_d[128 * c : 128 * (c + 1), :],
            )
            c2b_q.append(t)

        for c in range(min(PRE, NCH)):
            issue_c2b(c)

        # ---------- x transposes ----------
        xT = []
        for hc in range(NHC):
            ps = pp_tr.tile([128, L], BF16, tag="trp", name="psT")
            for jt in range(NJB):
                nc.tensor.transpose(
                    out=ps[:, 128 * jt : 128 * (jt + 1)],
                    in_=xf[jt][:, 128 * hc : 128 * (hc + 1)],
                    identity=identb,
                )
            xs = P.tile([128, L], BF16, tag=f"xT{hc}")
            nc.vector.tensor_copy(out=xs, in_=ps)
            xT.append(xs)
        xqT = []
        for hc in range(NHC):
            ps = pp_tr.tile([128, L], BF16, tag="trp", name="psT")
            for ib in range(NIB):
                nc.tensor.transpose(
                    out=ps[:, IBS * ib : IBS * (ib + 1)],
                    in_=xqb_sb[ib][:, 128 * hc : 128 * (hc + 1)],
                    identity=identb[0:IBS, 0:IBS],
                )
            xs = P.tile([128, LI], BF16, tag=f"xqT{hc}")
            nc.vector.tensor_copy(out=xs, in_=ps[:, 0:LI])
            xqT.append(xs)

        # ---------- q/k/v projections (bf16) ----------
        qT = []
        for t in range(NHC):
            ps = pp_big.tile([128, BIGF], F32, tag="big", name="psq")
            for kk in range(NHC):
                nc.tensor.matmul(
                    out=ps[:, 0:LI],
                    lhsT=w_sb("wq", kk)[:, 128 * t : 128 * (t + 1)],
                    rhs=xqT[kk], start=(kk == 0), stop=(kk == NHC - 1),
                )
            s = P.tile([128, LI], BF16, tag=f"qT{t}")
            if has_bq:
                nc.scalar.activation(out=s, in_=ps[:, 0:LI], func=AF.Identity,
                                     bias=bias_sb[("bq", t)])
            else:
                nc.scalar.copy(out=s, in_=ps[:, 0:LI])
            qT.append(s)
        kT = []
        for t in range(NHC):
            ps = pp_big.tile([128, BIGF], F32, tag="big", name="psk")
            for kk in range(NHC):
                nc.tensor.matmul(
                    out=ps[:, 0:L],
                    lhsT=w_sb("wk", kk)[:, 128 * t : 128 * (t + 1)],
                    rhs=xT[kk], start=(kk == 0), stop=(kk == NHC - 1),
                )
            s = P.tile([128, L], BF16, tag=f"kT{t}")
            if has_bk:
                nc.scalar.activation(out=s, in_=ps[:, 0:L], func=AF.Identity,
                                     bias=bias_sb[("bk", t)])
            else:
                nc.vector.tensor_copy(out=s, in_=ps[:, 0:L])
            kT.append(s)
        v_sb = []
        for t in range(NJB):
            ps = pp_big.tile([128, BIGF], F32, tag="big", name="psv")
            for kk in range(NHC):
                nc.tensor.matmul(
                    out=ps[:, 0:H],
                    lhsT=xT[kk][:, 128 * t : 128 * (t + 1)],
                    rhs=w_sb("wv", kk),
                    start=(kk == 0), stop=(kk == NHC - 1),
                )
            s = P.tile([128, H], BF16, tag=f"v{t}")
            nc.scalar.copy(out=s, in_=ps[:, 0:H])  # bv folded into avT evac
            v_sb.append(s)

        refS = [
            P.tile([IBS, NH, L], F32, tag=f"refS{ib}", name=f"refS{ib}")
            for ib in range(NIB)
        ]
        aoT = [P.tile([128, LI], BF16, tag=f"aoT{t}", name=f"aoT{t}")
               for t in range(NHC)]

        st = {"l1p": None, "l2p": None}
        pending_deint = []

        def flush_deints():
            # split between GPSIMD (SWDGE) and SP (HWDGE) so neither device
            # becomes the bottleneck; deferral keeps SP head-of-line safe
            while pending_deint:
                ib, u, l2s = pending_deint.pop(0)
                for nh in range(NH):
                    eng = nc.gpsimd if (u + nh) % 2 == 0 else nc.sync
                    eng.dma_start(
                        out=refS[ib][16 * u : 16 * u + 16, nh, :],
                        in_=l2s[nh : 128 : 8, :],
                    )

        def emit_chunk(c):
            """16 i-rows: prefetch DMA + deferred deint + transposes + MLP."""
            ib = (16 * c) // IBS
            if c + PRE < NCH:
                issue_c2b(c + PRE)
            flush_deints()
            c2b = c2b_q.popleft()
            for pr in range(8):
                r = 8 * c + pr          # global pair
                m, sub = r // 2, r % 2  # 2-pair unit
                trp = pp_tr.tile([2 * CIN, L], BF16, tag="trp")
                for jb in range(NJB):
                    nc.tensor.transpose(
                        out=trp[:, 128 * jb : 128 * (jb + 1)],
                        in_=c2b[:, pr, jb, :, :].rearrange("p i c -> p (i c)"),
                        identity=identb,
                    )
                trs = ptrs.tile([2 * CIN, L], BF16, tag="trs")
                nc.vector.tensor_copy(out=trs, in_=trp)
                if sub == 0:
                    st["l1p"] = pp_l1.tile([128, BIGF], F32, tag="l1", name="l1p")[:, 0:L]
                nc.tensor.matmul(
                    out=st["l1p"][64 * sub : 64 * sub + 64, :],
                    lhsT=bd1b, rhs=trs, start=True, stop=True,
                )
                if sub == 1:
                    hid = phid.tile([128, L], BF16, tag="hid")
                    nc.scalar.activation(
                        out=hid, in_=st["l1p"], func=AF.Relu, bias=r1b4,
                    )
                    q4 = m % 4
                    if q4 == 0:
                        st["l2p"] = pp_l2.tile([128, L], F32, tag="l2", name="l2p")
                    nc.tensor.matmul(
                        out=st["l2p"][32 * q4 : 32 * q4 + 32, :],
                        lhsT=bd2b, rhs=hid, start=True, stop=True,
                        tile_position=(0, 32 * q4),
                    )
                    if q4 == 3:
                        u = (m // 4) % (IBS // 16)
                        l2s = pl2s.tile([128, L], F32, tag="l2s")
                        if has_r2b:
                            nc.vector.tensor_scalar_add(
                                out=l2s, in0=st["l2p"], scalar1=r2b128,
                            )
                            nc.vector.tensor_add(out=l2s, in0=l2s, in1=mbias128)
                        else:
                            nc.vector.tensor_add(out=l2s, in0=st["l2p"], in1=mbias128)
                        pending_deint.append((ib, u, l2s))

        def emit_head(ib, nh, alt=False):
            t, s = nh // 2, nh % 2
            odd = alt and (nh % 2 == 1)
            sp = (pp_l1 if odd else pp_big).tile(
                [IBS, BIGF], F32, tag=("l1" if odd else "big"), name="sp")[:, 0:L]
            nc.tensor.matmul(
                out=sp,
                lhsT=qT[t][64 * s : 64 * s + 64, IBS * ib : IBS * (ib + 1)],
                rhs=kT[t][64 * s : 64 * s + 64, :],
                start=True, stop=False,
            )
            nc.tensor.matmul(
                out=sp, lhsT=ones1b[:, 0:IBS], rhs=jbias_b,
                start=False, stop=True, skip_group_check=True,
            )
            ea_t = pea.tile([IBS, L], BF16, tag="ea")
            sa = psc.tile([IBS, 1], F32, tag="sa")
            nc.scalar.activation(
                out=ea_t, in_=sp, func=AF.Exp, bias=0.0, scale=scale, accum_out=sa
            )
            er_t = per.tile([IBS, L], BF16, tag="er")
            sr = psc.tile([IBS, 1], F32, tag="sr")
            nc.scalar.activation(
                out=er_t, in_=refS[ib][:, nh, :], func=AF.Exp, bias=0.0, scale=1.0,
                accum_out=sr,
            )
            isa = psc.tile([IBS, 1], F32, tag="isa")
            nc.vector.reciprocal(out=isa, in_=sa)
            isr = psc.tile([IBS, 1], F32, tag="isr")
            nc.vector.reciprocal(out=isr, in_=sr)
            # w = ea/sa + er/sr (0.5 folded into avT evac); invalid-i -> 2/L
            nc.vector.tensor_scalar_mul(out=ea_t, in0=ea_t, scalar1=isa)
            nc.vector.scalar_tensor_tensor(
                out=ea_t, in0=er_t, scalar=isr, in1=ea_t, op0=ALU.mult, op1=ALU.add
            )
            nc.vector.tensor_scalar(
                out=ea_t, in0=ea_t, scalar1=miof[ib][:, 0:1], scalar2=miof[ib][:, 1:2],
                op0=ALU.mult, op1=ALU.add,
            )
            wtp = (pp_tr if odd else pp_wav).tile(
                [128, NJB * IBS], BF16, tag=("trp" if odd else "wav"), name="wtp")
            for k in range(NJB):
                nc.tensor.transpose(
                    out=wtp[:, IBS * k : IBS * (k + 1)],
                    in_=ea_t[:, 128 * k : 128 * (k + 1)],
                    identity=identb[0:IBS, 0:IBS],
                )
            wts = pwts.tile([128, NJB * IBS], BF16, tag="wts")
            nc.vector.tensor_copy(out=wts, in_=wtp)
            avp = (pp_tr if odd else pp_wav).tile(
                [64, IBS], F32, tag=("trp" if odd else "wav"), name="avp")
            for k in range(NJB):
                nc.tensor.matmul(
                    out=avp,
                    lhsT=v_sb[k][:, 64 * nh : 64 * nh + 64],
                    rhs=wts[:, IBS * k : IBS * (k + 1)],
                    start=(k == 0), stop=(k == NJB - 1),
                )
            if has_bv:
                nc.scalar.activation(
                    out=aoT[t][64 * s : 64 * s + 64, IBS * ib : IBS * (ib + 1)],
                    in_=avp, func=AF.Identity, scale=0.5,
                    bias=bias_sb[("bv", t)][64 * s : 64 * s + 64, :],
                )
            else:
                nc.scalar.activation(
                    out=aoT[t][64 * s : 64 * s + 64, IBS * ib : IBS * (ib + 1)],
                    in_=avp, func=AF.Copy, bias=0.0, scale=0.5,
                )

        def rsqrt_dve(out, v):
            """out = 1/sqrt(v) on DVE only (magic seed + Newton steps)."""
            yb = psc.tile([IBS, 1], I32, tag="rsq_i")
            nc.vector.tensor_scalar(
                out=yb, in0=v.bitcast(I32), scalar1=1, scalar2=None,
                op0=ALU.logical_shift_right,
            )
            nc.vector.tensor_scalar(
                out=yb, in0=yb, scalar1=-1, scalar2=0x5F3759DF,
                op0=ALU.mult, op1=ALU.add,
            )
            y = yb.bitcast(F32)
            t2 = psc.tile([IBS, 1], F32, tag="rsq_t")
            for _ in range(3):
                nc.vector.tensor_mul(out=t2, in0=y, in1=y)
                nc.vector.tensor_mul(out=t2, in0=t2, in1=v)
                nc.vector.tensor_scalar(
                    out=t2, in0=t2, scalar1=-0.5, scalar2=1.5,
                    op0=ALU.mult, op1=ALU.add,
                )
                nc.vector.tensor_mul(out=y, in0=y, in1=t2)
            nc.vector.tensor_copy(out=out, in_=y)

        def emit_proj(ib):
            pp = pp_big.tile([IBS, BIGF], F32, tag="big", name="pp")
            for kk in range(NHC):
                nc.tensor.matmul(
                    out=pp[:, 0:H],
                    lhsT=aoT[kk][:, IBS * ib : IBS * (ib + 1)],
                    rhs=w_sb("wo", kk),
                    start=(kk == 0), stop=(kk == NHC - 1),
                )
            r_t = pry.tile([IBS, H], F32, tag="rt")
            nc.vector.scalar_tensor_tensor(
                out=r_t, in0=pp[:, 0:H], scalar=1.0, in1=xq_sb[ib],
                op0=ALU.mult, op1=ALU.add,
            )
            if has_bo:
                nc.vector.tensor_add(out=r_t, in0=r_t, in1=bo_bc[0:IBS, :])
            stats = psc.tile([IBS, 6], F32, tag="stats")
            nc.vector.bn_stats(out=stats, in_=r_t)
            mv = psc.tile([IBS, 2], F32, tag="mv")
            nc.vector.bn_aggr(out=mv, in_=stats)
            veps = psc.tile([IBS, 1], F32, tag="veps")
            nc.vector.tensor_scalar_add(out=veps, in0=mv[:, 1:2], scalar1=1e-5)
            rstd = psc.tile([IBS, 1], F32, tag="rstd")
            rsqrt_dve(rstd, veps)
            nc.vector.tensor_scalar(
                out=r_t, in0=r_t, scalar1=mv[:, 0:1], scalar2=rstd,
                op0=ALU.subtract, op1=ALU.mult,
            )
            y_t = pry.tile([IBS, H], F32, tag="yt")
            nc.vector.tensor_mul(out=y_t, in0=r_t, in1=g_bc[0:IBS, :])
            nc.vector.tensor_add(out=y_t, in0=y_t, in1=b_bc[0:IBS, :])
            nc.scalar.dma_start(out=y_d[IBS * ib : IBS * (ib + 1), :], in_=y_t)

        # ---------- schedule ----------
        for c in range(CPI):
            emit_chunk(c)
        for ib in range(NIB):
            nxt = list(range(CPI * (ib + 1), min(CPI * (ib + 2), NCH)))
            flush_deints()  # refS[ib] writes must precede head reads
            alt = not nxt
            per_head = (len(nxt) + NH - 1) // NH if nxt else 0
            for nh in range(NH):
                emit_head(ib, nh, alt=alt)
                for _ in range(per_head):
                    if nxt:
                        emit_chunk(nxt.pop(0))
            while nxt:
                emit_chunk(nxt.pop(0))
            emit_proj(ib)
        flush_deints()

    nc.compile()
    return nc


def _make_bd1(r1w):
    bd1 = np.zeros((2 * CIN, 2 * CHID), np.float32)
    bd1[0:CIN, 0:CHID] = r1w
    bd1[CIN : 2 * CIN, CHID : 2 * CHID] = r1w
    return bd1


def _make_bd2(r2w):
    bd2 = np.zeros((128, 4 * NH), np.float32)
    for g in range(4):
        bd2[32 * g : 32 * g + CHID, NH * g : NH * (g + 1)] = r2w
    return bd2


def _make_r1b4(r1b):
    r1b4 = np.zeros((128, 1), np.float32)
    for g in range(4):
        r1b4[32 * g : 32 * g + CHID, 0] = r1b
    return r1b4


def _pack_refc(rc, L):
    """[LI, L, CIN] f32 -> [(LI/16)*128, 8*NJB*2*CIN] bf16 in c2b tile layout."""
    LI = rc.shape[0]
    NJB = L // 128
    nch = LI // 16
    a = rc.reshape(nch, 8, 2, 128, NJB, CIN)   # (c, i2, i, p, k, cc)
    a = a.transpose(0, 3, 1, 4, 2, 5)          # (c, p, i2, k, i, cc)
    return np.ascontiguousarray(
        a.reshape(nch * 128, 8 * NJB * 2 * CIN).astype(BF)
    )


_PROG_CACHE = {}


def _get_program(L, LI, flags, r2b_vals):
    key = (L, LI, flags)
    if key not in _PROG_CACHE:
        _PROG_CACHE[key] = build_program(L, LI, *flags, r2b_vals)
    return _PROG_CACHE[key]


def make_in_maps(x, mask, refCov, wq, bq, wk, bk, wv, bv, wo, bo,
                 r1w, r1b, r2w, r2b, ln_g, ln_b, n_cores=N_CORES, LI=None):
    Bc, L, Hc = x.shape
    if LI is None:
        LI = (Bc * L) // n_cores
    f = np.float32
    shared = {
        "wqkvo": np.ascontiguousarray(
            np.concatenate([np.asarray(w, f) for w in (wq, wk, wv, wo)], axis=1)
        ).astype(BF),
        "bqkvo": np.ascontiguousarray(
            np.stack([np.asarray(b, f) for b in (bq, bk, bv, bo)], axis=1)
        ),
        "bd1h": _make_bd1(np.asarray(r1w, f)).astype(BF),
        "bd2h": _make_bd2(np.asarray(r2w, f)).astype(BF),
        "r1b4h": _make_r1b4(np.asarray(r1b, f)),
        "lng": np.ascontiguousarray(ln_g, f).reshape(Hc, 1),
        "lnb": np.ascontiguousarray(ln_b, f).reshape(Hc, 1),
    }
    njb = L // 128
    pidx = (np.arange(L) % 128) * njb + np.arange(L) // 128
    per_batch = L // LI
    in_maps = []
    for c in range(n_cores):
        b, half = c // per_batch, c % per_batch
        i0 = half * LI
        m = dict(shared)
        xb = np.asarray(x[b], f)
        m["xb"] = np.ascontiguousarray(xb).astype(BF)
        m["xqb"] = np.ascontiguousarray(xb[i0 : i0 + LI]).astype(BF)
        m["xq"] = np.ascontiguousarray(xb[i0 : i0 + LI])
        m["refcb"] = _pack_refc(np.asarray(refCov[b, i0 : i0 + LI], f), L)
        mp = np.asarray(mask[b][pidx], f)          # permuted j-mask (1 valid)
        m["jbias"] = np.ascontiguousarray((NEG * (1.0 - mp)).reshape(1, L)).astype(BF)
        m["mbias"] = np.ascontiguousarray((NEG * (1.0 - mp)).reshape(L, 1))
        mi = np.asarray(mask[b, i0 : i0 + LI], f)
        m["miof"] = np.ascontiguousarray(
            np.stack([mi, (1.0 - mi) * (2.0 / L)], axis=1)
        )
        in_maps.append(m)
    return in_maps, per_batch, LI


def kernel(x, mask, refCov, wq, bq, wk, bk, wv, bv, wo, bo,
           r1w, r1b, r2w, r2b, ln_g, ln_b, trace=False):
    x = np.asarray(x)
    Bc, L, Hc = x.shape
    flags = (
        bool(np.any(bq)), bool(np.any(bk)), bool(np.any(bv)), bool(np.any(bo)),
        bool(np.any(r2b)),
    )
    in_maps, per_batch, LI = make_in_maps(
        x, mask, refCov, wq, bq, wk, bk, wv, bv, wo, bo,
        r1w, r1b, r2w, r2b, ln_g, ln_b,
    )
    nc = _get_program(L, LI, flags, [float(v) for v in np.asarray(r2b).ravel()])
    res = run_bass_kernel_spmd(nc, in_maps, core_ids=list(range(N_CORES)), trace=trace)
    out = np.empty((Bc, L, Hc), np.float32)
    for c in range(N_CORES):
        b, half = c // per_batch, c % per_batch
        out[b, half * LI : (half + 1) * LI] = res.results[c]["y"]
    if trace:
        return out, res
    return out
